# revision 1
# baseline (speedup 1.0000x reference)
import numpy as np
import jax
import jax.numpy as jnp
from jax.sharding import Mesh, PartitionSpec as P
from jax.experimental.shard_map import shard_map
from functools import partial

# Problem constants (hardcoded per spec)
B, L, D, N = 4, 4096, 1024, 512
LN_EPS = 1e-5
CH = 128          # chunk length
NC = L // CH      # 32 chunks


_IDX = np.arange(CH)[:, None] - np.arange(CH)[None, :]
_MASK = (_IDX >= 0)[:, :, None]
_IDXC = np.clip(_IDX, 0, CH - 1)


def _dss_shard(x, A1r, A1i, EPr, EPi, E2r, E2i, Ctr, Cti, Kloc, Dh, g, bta, didx):
    # build per-d triangular toeplitz on device from Kloc [CH, Dh]
    Ttoe = jnp.where(_MASK, Kloc[_IDXC, :], 0.0)
    # x: [1, L, D] full-channel batch shard; everything else local d-half (Dh=512)
    xb = x[0]                                    # [L, D]
    mu = jnp.mean(xb, axis=-1, keepdims=True)
    var = jnp.var(xb, axis=-1, keepdims=True)
    un = (xb - mu) * jax.lax.rsqrt(var + LN_EPS) * g + bta   # [L, D]
    h = jax.lax.axis_index('h')
    u = jax.lax.dynamic_slice(un, (0, h * (D // 2)), (L, D // 2))  # [L, 512]

    uc = u.reshape(NC, CH, D // 2)               # [c, s, d]
    # local (per-chunk) states: Sloc[c,n,d] = sum_s A1[s,n] * u[c,s,d]
    Slr = jnp.einsum('sn,csd->cnd', A1r, uc)
    Sli = jnp.einsum('sn,csd->cnd', A1i, uc)

    # scan over chunks: S[c] = EP*S[c-1] + Sloc[c-1]  (complex diag per n)
    def step(carry, sl):
        sr, si = carry
        slr, sli = sl
        nsr = EPr[:, None] * sr - EPi[:, None] * si + slr
        nsi = EPr[:, None] * si + EPi[:, None] * sr + sli
        return (nsr, nsi), (sr, si)
    z = jnp.zeros((N, D // 2), jnp.float32)
    try:
        z = jax.lax.pcast(z, ('b', 'h'), to='varying')
    except AttributeError:
        z = jax.lax.pvary(z, ('b', 'h'))
    _, (Spr, Spi) = jax.lax.scan(step, (z, z), (Slr, Sli))
    # Spr[c] = state BEFORE chunk c? scan emits carry before update, with inputs
    # Sloc[c]: emitted carry at step c is S after chunks < c... check: at step c,
    # emit (sr,si) = state from chunks [0..c-1] then update with Sloc[c]. Correct.

    # W = Ct (conj layout [n,d]) hadamard S
    Wr = Ctr * Spr - Cti * Spi
    Wi = Ctr * Spi + Cti * Spr

    # inter-chunk output: y_int[c,t,d] = Re sum_n E2[t,n] W[c,n,d]
    y_int = jnp.einsum('tn,cnd->ctd', E2r, Wr) - jnp.einsum('tn,cnd->ctd', E2i, Wi)

    # intra-chunk causal: y_intra[c,t,d] = sum_{s<=t} Ttoe[t,s,d] u[c,s,d]
    y_intra = jnp.einsum('tsd,csd->ctd', Ttoe, uc)

    y = (y_int + y_intra).reshape(L, D // 2) + u * Dh[None, :]
    return y[None]                               # [1, L, 512]


def kernel(x, Lambda_real, Lambda_imag, C_real, C_imag, param_D, ln_gamma, ln_beta):
    x = np.asarray(x, np.float32)
    # ---- host precompute in float64 ----
    Lr = -np.exp(np.asarray(Lambda_real, np.float64))
    Li = np.exp(np.asarray(Lambda_imag, np.float64))
    lam = Lr + 1j * Li                                    # [N]
    Cc = (np.asarray(C_real, np.float64) + 1j * np.asarray(C_imag, np.float64))
    Ct = Cc * (np.exp(lam) - 1.0) / lam                   # [D, N]

    s = np.arange(CH)
    A1 = np.exp(lam[None, :] * (CH - 1 - s)[:, None])     # [s, n] e^{lam*(CH-1-s)}
    EP = np.exp(lam * CH)                                 # [n]
    t = np.arange(CH)
    E2 = np.exp(lam[None, :] * (t + 1)[:, None])          # [t, n]
    # intra toeplitz per d-half later; K_loc[tau, d] = Re sum_n Ct[d,n] e^{lam tau}
    tau = np.arange(CH)
    Etau = np.exp(lam[None, :] * tau[:, None])            # [tau, n]
    Kloc = np.real(Etau @ Ct.T)                           # [tau, D]

    f32 = lambda a: np.ascontiguousarray(np.real(a), np.float32)
    A1r, A1i = f32(A1), np.ascontiguousarray(np.imag(A1), np.float32)
    EPr, EPi = f32(EP), np.ascontiguousarray(np.imag(EP), np.float32)
    E2r, E2i = f32(E2), np.ascontiguousarray(np.imag(E2), np.float32)
    # Ct in [n, d] layout per half
    CtT = Ct.T                                            # [N, D]
    Ctr = np.ascontiguousarray(np.real(CtT), np.float32)
    Cti = np.ascontiguousarray(np.imag(CtT), np.float32)
    KlocT = np.ascontiguousarray(Kloc, np.float32)        # [CH, D]
    Dv = np.asarray(param_D, np.float32)
    g = np.asarray(ln_gamma, np.float32)
    bta = np.asarray(ln_beta, np.float32)

    mesh, fn, specs = _get_fn()
    didx = np.zeros((), np.int32)
    args = (x, A1r, A1i, EPr, EPi, E2r, E2i, Ctr, Cti, KlocT, Dv, g, bta, didx)
    from jax.sharding import NamedSharding
    dargs = [jax.device_put(a, NamedSharding(mesh, sp)) for a, sp in zip(args, specs)]
    y = fn(*dargs)
    return np.asarray(jax.device_get(y), np.float32)


_CACHE = {}


def _get_fn():
    if 'fn' not in _CACHE:
        devs = np.array(jax.devices()[:8]).reshape(4, 2)
        mesh = Mesh(devs, ('b', 'h'))
        specs = (P('b', None, None), P(), P(), P(), P(), P(), P(),
                 P(None, 'h'), P(None, 'h'), P(None, 'h'), P('h'), P(), P(), P())
        fn = jax.jit(shard_map(_dss_shard, mesh=mesh, in_specs=specs,
                               out_specs=P('b', None, 'h')))
        _CACHE['fn'] = (mesh, fn, specs)
    return _CACHE['fn']

